# revision 44
# baseline (speedup 1.0000x reference)
"""1-D nearest-neighbor retrieval kernel for Trainium2 (8 NeuronCores).

For each query x[b], finds argmin_n |input_tensor[n] - x[b]| and returns
accuracy_tensor[argmin].  Queries are sharded across the 8 cores (512 each,
held as [128 partitions x 4 columns], query j -> partition j//4, column j%4
so the query load / result store move 16 contiguous bytes per partition);
the index table is replicated.

Instead of the O(B*N) brute-force distance sweep, the host builds a
uniform-grid index over the sorted reference points (standard offline index
build for retrieval) and the device runs an exact one-gather search:

  1. cell = u32(clamp((x - lo) * scale, 0, G-1)) -- three tiny ops, no scan.
     The host mirrors the fp32 subtract/multiply/clamp exactly; the table
     row windows are built to cover the insertion range for ANY monotone
     fp32->int cast with floor(v) <= cast(v) <= ceil(v), so the device's
     rounding mode is irrelevant.
  2. One indirect-DMA gather of the 512-byte row:
        row = [ s-window (33) | accuracies (33) | tie-accs (32) | pad ]
     where the s-window holds sorted refs s[gmin-1 .. gmin+WN-1] with
     gmin = #{refs r : t(r) <= cell-1} (so window[0] < x always), +-BIG
     sentinels past the array ends, acc = run-first accuracy (duplicate
     -value runs pre-resolved to the run's first-original-index accuracy),
     tie-acc = per adjacent pair, the accuracy of the side whose value-run
     has the smaller first original index.
  3. wcnt = #{window refs < x} (one fused compare+accumulate); jL = wcnt-1
     and jR = wcnt index the sorted neighbors L < x <= R.  Five fused
     one-instruction masked-sum extractions ((iota==wcnt)*field, exact:
     sums of one nonzero value and zeros) pull sL, sR, accL, accR, accTie.
  4. dL = x - L and dR = R - x (bit-identical to |ref - x| in fp32); pick
     aR if dR < dL, accTie if dR == dL (matching jnp.argmin's
     first-occurrence tie-break exactly), else aL -- via exact {0,1} masks.

All comparisons/counts are exact fp32 integer arithmetic, so the result
matches the jax reference bit-for-bit, including all argmin tie-breaks
(verified exhaustively against the full O(B*N) distance matrix).
"""
from contextlib import ExitStack

import numpy as np

import concourse.bass as bass
import concourse.bacc as bacc
import concourse.tile as tile
from concourse import mybir
from concourse._compat import with_exitstack
from concourse.bass_utils import run_bass_kernel_spmd

P = 128
N_CORES = 8
B = 4096
B_CORE = B // N_CORES  # 512
Q = B_CORE // P        # 4 query columns per core
N = 65536
G = 32768              # uniform grid cells
WN = 32                # window entries per row (max insertion span + 2 <= WN)
RW = 128               # row stride in floats (512 B)
BIG = np.float32(3.0e38)

FP32 = mybir.dt.float32
U32 = mybir.dt.uint32

ALU = mybir.AluOpType


@with_exitstack
def _nn_kernel(
    ctx: ExitStack, tc: tile.TileContext, xq, grid, out, lo, scale
):
    nc = tc.nc
    pool = ctx.enter_context(tc.tile_pool(name="p", bufs=1))

    x_sb = pool.tile([P, Q], FP32, tag="x_sb")
    nc.sync.dma_start(out=x_sb[:], in_=xq.rearrange("(p q) -> p q", q=Q))
    # One-hot compare values j+1 (so the one-hot compares vs wcnt directly,
    # no -1 step), generated on-chip: iota is integer-only, so cast after.
    io_i = pool.tile([P, WN], mybir.dt.int32, tag="io_i")
    nc.gpsimd.iota(io_i[:], pattern=[[1, WN]], base=1, channel_multiplier=0)
    io_bc = pool.tile([P, WN], FP32, tag="io_bc")
    nc.vector.tensor_copy(io_bc[:], io_i[:])

    # cell = u32(clamp((x - lo) * scale, 0, G-1));  lo/scale are baked-in
    # fp32 immediates (mirrored exactly by the host table build).
    c1 = pool.tile([P, Q], FP32, tag="c1")
    nc.vector.tensor_scalar(
        c1[:], x_sb[:], float(lo), float(scale), op0=ALU.subtract, op1=ALU.mult
    )
    c2 = pool.tile([P, Q], FP32, tag="c2")
    nc.vector.tensor_scalar(c2[:], c1[:], 0.0, float(G - 1), op0=ALU.max, op1=ALU.min)
    cellu = pool.tile([P, Q], U32, tag="cellu")
    nc.vector.tensor_copy(cellu[:], c2[:])

    # One 512B-row gather per query column (HW honors one offset/partition).
    rows = pool.tile([P, Q * RW], FP32, tag="rows")
    for q in range(Q):
        nc.gpsimd.indirect_dma_start(
            out=rows[:, q * RW : (q + 1) * RW],
            out_offset=None,
            in_=grid,
            in_offset=bass.IndirectOffsetOnAxis(ap=cellu[:, q : q + 1], axis=0),
        )

    wc4 = pool.tile([P, Q], FP32, tag="wc4")
    scr = pool.tile([P, Q * WN], FP32, tag="scr")
    sL4 = pool.tile([P, Q], FP32, tag="sL4")
    sR4 = pool.tile([P, Q], FP32, tag="sR4")
    aL4 = pool.tile([P, Q], FP32, tag="aL4")
    aR4 = pool.tile([P, Q], FP32, tag="aR4")
    aT4 = pool.tile([P, Q], FP32, tag="aT4")

    # Row layout: [ s (WN+1) | af (WN+1) | aT (WN) | pad ].  The R views are
    # the same fields shifted one entry, which is why s/af carry WN+1 values.
    for q in range(Q):
        base = q * RW
        s_part = rows[:, base : base + WN]
        xcol = x_sb[:, q : q + 1]
        scrq = scr[:, q * WN : (q + 1) * WN]
        # wcnt = #{window < x}.  window[0] < x by construction, so
        # jL = wcnt-1 >= 0; the one-hot iota holds j+1 so the fused
        # extractions compare it against wcnt directly.
        nc.vector.tensor_scalar(
            scrq, s_part, xcol, 0.0,
            op0=ALU.is_lt, op1=ALU.add, accum_out=wc4[:, q : q + 1],
        )
        # One-instruction masked-sum extractions: (iota == wcnt) * field,
        # accumulated.  Exact: sums of one nonzero value and zeros.
        # aT is the host-precomputed tie-winner accuracy of the (jL, jR)
        # pair (the side whose value-run has the smaller first original
        # index), used when dR == dL exactly.
        for dst, view in (
            (sL4, rows[:, base : base + WN]),
            (sR4, rows[:, base + 1 : base + 1 + WN]),
            (aL4, rows[:, base + WN + 1 : base + 2 * WN + 1]),
            (aR4, rows[:, base + WN + 2 : base + 2 * WN + 2]),
            (aT4, rows[:, base + 2 * WN + 2 : base + 3 * WN + 2]),
        ):
            nc.vector.scalar_tensor_tensor(
                scrq, io_bc[:], wc4[:, q : q + 1], view,
                op0=ALU.is_equal, op1=ALU.mult, accum_out=dst[:, q : q + 1],
            )

    def tt(name, a, b_, op):
        t = pool.tile([P, Q], FP32, tag=name)
        nc.vector.tensor_tensor(out=t[:], in0=a, in1=b_, op=op)
        return t

    dL = tt("dL", x_sb[:], sL4[:], ALU.subtract)    # x - L  (= |L - x|, exact)
    dR = tt("dR", sR4[:], x_sb[:], ALU.subtract)    # R - x  (= |R - x|, exact)
    t1 = tt("t1", dR[:], dL[:], ALU.is_lt)          # dR < dL   -> pick aR
    t2 = tt("t2", dR[:], dL[:], ALU.is_equal)       # dR == dL  -> pick aT
    s12 = tt("s12", t1[:], t2[:], ALU.add)
    nsel = pool.tile([P, Q], FP32, tag="nsel")      # else      -> pick aL
    nc.vector.tensor_scalar(
        nsel[:], s12[:], -1.0, 1.0, op0=ALU.mult, op1=ALU.add
    )
    m1 = tt("m1", t1[:], aR4[:], ALU.mult)          # exact: masks in {0,1}
    m2 = tt("m2", t2[:], aT4[:], ALU.mult)
    m3 = tt("m3", nsel[:], aL4[:], ALU.mult)
    o1 = tt("o1", m1[:], m2[:], ALU.add)
    outv = tt("outv", o1[:], m3[:], ALU.add)

    nc.sync.dma_start(out=out.rearrange("(p q) -> p q", q=Q), in_=outv[:])


_CACHED_NC = {}


def _build(lo, scale):
    key = (float(lo), float(scale))
    if key in _CACHED_NC:
        return _CACHED_NC[key]
    nc = bacc.Bacc("TRN2", target_bir_lowering=False, debug=False)
    xq = nc.dram_tensor("xq", [B_CORE], FP32, kind="ExternalInput").ap()
    grid = nc.dram_tensor("grid", [G, RW], FP32, kind="ExternalInput").ap()
    out = nc.dram_tensor("out", [B_CORE], FP32, kind="ExternalOutput").ap()
    with tile.TileContext(nc) as tc:
        _nn_kernel(tc, xq, grid, out, lo, scale)
    nc.compile()
    _CACHED_NC[key] = nc
    return nc


def _build_tables(refs, acc):
    """Sorted refs + uniform-grid window table. Exact, including ties.

    Windows are sized for any monotone fp32->int cast between floor and
    ceil, so the device's cast rounding mode does not matter.
    """
    order = np.argsort(refs, kind="stable")
    s = refs[order]
    # First original index / accuracy of each equal-value run (stable sort
    # puts the smallest original index first in each run).
    run_start = np.concatenate([[0], np.nonzero(np.diff(s) != 0)[0] + 1])
    run_id = np.zeros(N, dtype=np.int64)
    run_id[run_start] = 1
    run_id = np.cumsum(run_id) - 1
    head = order[run_start[run_id]]
    fi = head.astype(np.float32)
    af = acc[head]

    lo = np.float32(s[0])
    span = np.float32(np.float32(s[-1]) - lo)
    scale = np.float32(np.float32(np.float32(G) / span) * np.float32(0.999))

    # Mirror the device's fp32 (x - lo) * scale, clamp, exactly.
    t = ((s - lo) * scale).astype(np.float32)
    tS = np.minimum(np.maximum(t, np.float32(0.0)), np.float32(G - 1)).astype(
        np.float64
    )
    c = np.arange(G, dtype=np.float64)
    gmin = np.searchsorted(tS, c - 1, side="right")  # #{t(s) <= c-1}
    gmax = np.searchsorted(tS, c + 1, side="left")   # #{t(s) <  c+1}
    wmax = int((gmax - gmin).max()) + 2
    assert wmax <= WN, f"grid overflow: need WN >= {wmax}"

    # Per adjacent sorted pair (j, j+1): the accuracy of the side whose
    # value-run has the smaller first original index -- the exact argmin
    # winner when the two fp32 distances tie.
    at = np.where(fi[1:] < fi[:-1], af[1:], af[:-1]).astype(np.float32)

    def wfield(arr, lo_fill, hi_fill, width):
        # window field: position j of cell c -> arr[gmin[c]-1+j] with fills
        pmat = gmin[:, None] - 1 + np.arange(width)[None, :]
        v = np.where(
            pmat < 0,
            np.float32(lo_fill),
            np.where(
                pmat > len(arr) - 1, np.float32(hi_fill), arr[np.clip(pmat, 0, len(arr) - 1)]
            ),
        ).astype(np.float32)
        return v

    grid = np.zeros((G, RW), dtype=np.float32)
    grid[:, 0 : WN + 1] = wfield(s, -BIG, BIG, WN + 1)
    grid[:, WN + 1 : 2 * WN + 2] = wfield(af, 0.0, 0.0, WN + 1)
    grid[:, 2 * WN + 2 : 3 * WN + 2] = wfield(at, 0.0, 0.0, WN)

    return np.ascontiguousarray(grid), lo, scale


def kernel(x, input_tensor, accuracy_tensor):
    x = np.asarray(x, dtype=np.float32)
    refs = np.ascontiguousarray(np.asarray(input_tensor, dtype=np.float32))
    acc = np.ascontiguousarray(np.asarray(accuracy_tensor, dtype=np.float32))

    grid, lo, scale = _build_tables(refs, acc)
    nc = _build(lo, scale)
    in_maps = [
        {
            "xq": np.ascontiguousarray(x[i * B_CORE : (i + 1) * B_CORE]),
            "grid": grid,
        }
        for i in range(N_CORES)
    ]
    res = run_bass_kernel_spmd(nc, in_maps, core_ids=list(range(N_CORES)))
    return np.concatenate([res.results[i]["out"] for i in range(N_CORES)])


# revision 47
# speedup vs baseline: 1.0457x; 1.0457x over previous
"""1-D nearest-neighbor retrieval kernel for Trainium2 (8 NeuronCores).

For each query x[b], finds argmin_n |input_tensor[n] - x[b]| and returns
accuracy_tensor[argmin].  Queries are sharded across the 8 cores (512 each,
held as [128 partitions x 4 columns], query j -> partition j//4, column j%4
so the query load / result store move 16 contiguous bytes per partition);
the index table is replicated.

Instead of the O(B*N) brute-force distance sweep, the host builds a
uniform-grid index over the sorted reference points (standard offline index
build for retrieval) and the device runs an exact one-gather search:

  1. cell = u32(clamp((x - lo) * scale, 0, G-1)) -- three tiny ops, no scan.
     The host mirrors the fp32 subtract/multiply/clamp exactly; the table
     row windows are built to cover the insertion range for ANY monotone
     fp32->int cast with floor(v) <= cast(v) <= ceil(v), so the device's
     rounding mode is irrelevant.
  2. One indirect-DMA gather of the 512-byte row:
        row = [ s-window (33) | accuracies (33) | tie-accs (32) | pad ]
     where the s-window holds sorted refs s[gmin-1 .. gmin+WN-1] with
     gmin = #{refs r : t(r) <= cell-1} (so window[0] < x always), +-BIG
     sentinels past the array ends, acc = run-first accuracy (duplicate
     -value runs pre-resolved to the run's first-original-index accuracy),
     tie-acc = per adjacent pair, the accuracy of the side whose value-run
     has the smaller first original index.
  3. wcnt = #{window refs < x} (one fused compare+accumulate); jL = wcnt-1
     and jR = wcnt index the sorted neighbors L < x <= R.  Five fused
     one-instruction masked-sum extractions ((iota==wcnt)*field, exact:
     sums of one nonzero value and zeros) pull sL, sR, accL, accR, accTie.
  4. dL = x - L and dR = R - x (bit-identical to |ref - x| in fp32); pick
     aR if dR < dL, accTie if dR == dL (matching jnp.argmin's
     first-occurrence tie-break exactly), else aL -- via exact {0,1} masks.

All comparisons/counts are exact fp32 integer arithmetic, so the result
matches the jax reference bit-for-bit, including all argmin tie-breaks
(verified exhaustively against the full O(B*N) distance matrix).
"""
from contextlib import ExitStack

import numpy as np

import concourse.bass as bass
import concourse.bacc as bacc
import concourse.tile as tile
from concourse import mybir
from concourse._compat import with_exitstack
from concourse.bass_utils import run_bass_kernel_spmd

P = 128
N_CORES = 8
B = 4096
B_CORE = B // N_CORES  # 512
Q = B_CORE // P        # 4 query columns per core
N = 65536
G = 32768              # uniform grid cells
WN = 32                # window entries per row (max insertion span + 2 <= WN)
RW = 128               # row stride in floats (512 B)
BIG = np.float32(3.0e38)

FP32 = mybir.dt.float32
U32 = mybir.dt.uint32

ALU = mybir.AluOpType


@with_exitstack
def _nn_kernel(
    ctx: ExitStack, tc: tile.TileContext, xq, grid, out, lo, scale
):
    nc = tc.nc
    pool = ctx.enter_context(tc.tile_pool(name="p", bufs=1))

    x_sb = pool.tile([P, Q], FP32, tag="x_sb")
    nc.sync.dma_start(out=x_sb[:], in_=xq.rearrange("(p q) -> p q", q=Q))
    # One-hot compare values j+1 (so the one-hot compares vs wcnt directly,
    # no -1 step), generated on-chip: iota is integer-only, so cast after.
    io_i = pool.tile([P, WN], mybir.dt.int32, tag="io_i")
    nc.gpsimd.iota(io_i[:], pattern=[[1, WN]], base=1, channel_multiplier=0)
    io_bc = pool.tile([P, WN], FP32, tag="io_bc")
    nc.vector.tensor_copy(io_bc[:], io_i[:])

    # cell = u32(clamp((x - lo) * scale, 0, G-1));  lo/scale are baked-in
    # fp32 immediates (mirrored exactly by the host table build).
    c1 = pool.tile([P, Q], FP32, tag="c1")
    nc.vector.tensor_scalar(
        c1[:], x_sb[:], float(lo), float(scale), op0=ALU.subtract, op1=ALU.mult
    )
    c2 = pool.tile([P, Q], FP32, tag="c2")
    nc.vector.tensor_scalar(c2[:], c1[:], 0.0, float(G - 1), op0=ALU.max, op1=ALU.min)
    cellu = pool.tile([P, Q], U32, tag="cellu")
    nc.vector.tensor_copy(cellu[:], c2[:])

    # One 512B-row gather per query column (HW honors one offset/partition).
    # Separate tiles per column so each column's extraction only waits for
    # its own gather, keeping DVE work streaming behind the gather queue.
    rowt = [
        pool.tile([P, RW], FP32, tag=f"rows{q}", name=f"rows{q}")
        for q in range(Q)
    ]
    for q in range(Q):
        nc.gpsimd.indirect_dma_start(
            out=rowt[q][:],
            out_offset=None,
            in_=grid,
            in_offset=bass.IndirectOffsetOnAxis(ap=cellu[:, q : q + 1], axis=0),
        )

    wc4 = pool.tile([P, Q], FP32, tag="wc4")
    scr = pool.tile([P, Q * WN], FP32, tag="scr")
    sL4 = pool.tile([P, Q], FP32, tag="sL4")
    sR4 = pool.tile([P, Q], FP32, tag="sR4")
    aL4 = pool.tile([P, Q], FP32, tag="aL4")
    aR4 = pool.tile([P, Q], FP32, tag="aR4")
    aT4 = pool.tile([P, Q], FP32, tag="aT4")

    # Row layout: [ s (WN+1) | af (WN+1) | aT (WN) | pad ].  The R views are
    # the same fields shifted one entry, which is why s/af carry WN+1 values.
    for q in range(Q):
        rows = rowt[q]
        base = 0
        s_part = rows[:, base : base + WN]
        xcol = x_sb[:, q : q + 1]
        scrq = scr[:, q * WN : (q + 1) * WN]
        # wcnt = #{window < x}.  window[0] < x by construction, so
        # jL = wcnt-1 >= 0; the one-hot iota holds j+1 so the fused
        # extractions compare it against wcnt directly.
        nc.vector.tensor_scalar(
            scrq, s_part, xcol, 0.0,
            op0=ALU.is_lt, op1=ALU.add, accum_out=wc4[:, q : q + 1],
        )
        # One-instruction masked-sum extractions: (iota == wcnt) * field,
        # accumulated.  Exact: sums of one nonzero value and zeros.
        # aT is the host-precomputed tie-winner accuracy of the (jL, jR)
        # pair (the side whose value-run has the smaller first original
        # index), used when dR == dL exactly.
        for dst, view in (
            (sL4, rows[:, base : base + WN]),
            (sR4, rows[:, base + 1 : base + 1 + WN]),
            (aL4, rows[:, base + WN + 1 : base + 2 * WN + 1]),
            (aR4, rows[:, base + WN + 2 : base + 2 * WN + 2]),
            (aT4, rows[:, base + 2 * WN + 2 : base + 3 * WN + 2]),
        ):
            nc.vector.scalar_tensor_tensor(
                scrq, io_bc[:], wc4[:, q : q + 1], view,
                op0=ALU.is_equal, op1=ALU.mult, accum_out=dst[:, q : q + 1],
            )

    def tt(name, a, b_, op):
        t = pool.tile([P, Q], FP32, tag=name)
        nc.vector.tensor_tensor(out=t[:], in0=a, in1=b_, op=op)
        return t

    dL = tt("dL", x_sb[:], sL4[:], ALU.subtract)    # x - L  (= |L - x|, exact)
    dR = tt("dR", sR4[:], x_sb[:], ALU.subtract)    # R - x  (= |R - x|, exact)
    t1 = tt("t1", dR[:], dL[:], ALU.is_lt)          # dR < dL   -> pick aR
    t2 = tt("t2", dR[:], dL[:], ALU.is_equal)       # dR == dL  -> pick aT
    s12 = tt("s12", t1[:], t2[:], ALU.add)
    nsel = pool.tile([P, Q], FP32, tag="nsel")      # else      -> pick aL
    nc.vector.tensor_scalar(
        nsel[:], s12[:], -1.0, 1.0, op0=ALU.mult, op1=ALU.add
    )
    m1 = tt("m1", t1[:], aR4[:], ALU.mult)          # exact: masks in {0,1}
    m2 = tt("m2", t2[:], aT4[:], ALU.mult)
    m3 = tt("m3", nsel[:], aL4[:], ALU.mult)
    o1 = tt("o1", m1[:], m2[:], ALU.add)
    outv = tt("outv", o1[:], m3[:], ALU.add)

    nc.sync.dma_start(out=out.rearrange("(p q) -> p q", q=Q), in_=outv[:])


_CACHED_NC = {}


def _build(lo, scale):
    key = (float(lo), float(scale))
    if key in _CACHED_NC:
        return _CACHED_NC[key]
    nc = bacc.Bacc("TRN2", target_bir_lowering=False, debug=False)
    xq = nc.dram_tensor("xq", [B_CORE], FP32, kind="ExternalInput").ap()
    grid = nc.dram_tensor("grid", [G, RW], FP32, kind="ExternalInput").ap()
    out = nc.dram_tensor("out", [B_CORE], FP32, kind="ExternalOutput").ap()
    with tile.TileContext(nc) as tc:
        _nn_kernel(tc, xq, grid, out, lo, scale)
    nc.compile()
    _CACHED_NC[key] = nc
    return nc


def _build_tables(refs, acc):
    """Sorted refs + uniform-grid window table. Exact, including ties.

    Windows are sized for any monotone fp32->int cast between floor and
    ceil, so the device's cast rounding mode does not matter.
    """
    order = np.argsort(refs, kind="stable")
    s = refs[order]
    # First original index / accuracy of each equal-value run (stable sort
    # puts the smallest original index first in each run).
    run_start = np.concatenate([[0], np.nonzero(np.diff(s) != 0)[0] + 1])
    run_id = np.zeros(N, dtype=np.int64)
    run_id[run_start] = 1
    run_id = np.cumsum(run_id) - 1
    head = order[run_start[run_id]]
    fi = head.astype(np.float32)
    af = acc[head]

    lo = np.float32(s[0])
    span = np.float32(np.float32(s[-1]) - lo)
    scale = np.float32(np.float32(np.float32(G) / span) * np.float32(0.999))

    # Mirror the device's fp32 (x - lo) * scale, clamp, exactly.
    t = ((s - lo) * scale).astype(np.float32)
    tS = np.minimum(np.maximum(t, np.float32(0.0)), np.float32(G - 1)).astype(
        np.float64
    )
    c = np.arange(G, dtype=np.float64)
    gmin = np.searchsorted(tS, c - 1, side="right")  # #{t(s) <= c-1}
    gmax = np.searchsorted(tS, c + 1, side="left")   # #{t(s) <  c+1}
    wmax = int((gmax - gmin).max()) + 2
    assert wmax <= WN, f"grid overflow: need WN >= {wmax}"

    # Per adjacent sorted pair (j, j+1): the accuracy of the side whose
    # value-run has the smaller first original index -- the exact argmin
    # winner when the two fp32 distances tie.
    at = np.where(fi[1:] < fi[:-1], af[1:], af[:-1]).astype(np.float32)

    def wfield(arr, lo_fill, hi_fill, width):
        # window field: position j of cell c -> arr[gmin[c]-1+j] with fills
        pmat = gmin[:, None] - 1 + np.arange(width)[None, :]
        v = np.where(
            pmat < 0,
            np.float32(lo_fill),
            np.where(
                pmat > len(arr) - 1, np.float32(hi_fill), arr[np.clip(pmat, 0, len(arr) - 1)]
            ),
        ).astype(np.float32)
        return v

    grid = np.zeros((G, RW), dtype=np.float32)
    grid[:, 0 : WN + 1] = wfield(s, -BIG, BIG, WN + 1)
    grid[:, WN + 1 : 2 * WN + 2] = wfield(af, 0.0, 0.0, WN + 1)
    grid[:, 2 * WN + 2 : 3 * WN + 2] = wfield(at, 0.0, 0.0, WN)

    return np.ascontiguousarray(grid), lo, scale


def kernel(x, input_tensor, accuracy_tensor):
    x = np.asarray(x, dtype=np.float32)
    refs = np.ascontiguousarray(np.asarray(input_tensor, dtype=np.float32))
    acc = np.ascontiguousarray(np.asarray(accuracy_tensor, dtype=np.float32))

    grid, lo, scale = _build_tables(refs, acc)
    nc = _build(lo, scale)
    in_maps = [
        {
            "xq": np.ascontiguousarray(x[i * B_CORE : (i + 1) * B_CORE]),
            "grid": grid,
        }
        for i in range(N_CORES)
    ]
    res = run_bass_kernel_spmd(nc, in_maps, core_ids=list(range(N_CORES)))
    return np.concatenate([res.results[i]["out"] for i in range(N_CORES)])


# revision 49
# speedup vs baseline: 1.1343x; 1.0848x over previous
"""1-D nearest-neighbor retrieval kernel for Trainium2 (8 NeuronCores).

For each query x[b], finds argmin_n |input_tensor[n] - x[b]| and returns
accuracy_tensor[argmin].  Queries are sharded across the 8 cores (512 each,
held as [128 partitions x 4 columns]); the index table is replicated.

The host builds a uniform-grid index over the sorted reference points with
per-pair fp32 decision thresholds (standard offline index build), and the
device answers each query with ONE indirect gather + ONE count + ONE
masked-sum extraction:

  1. cell = u32(clamp((x - lo) * scale, 0, G-1)) -- three tiny ops.  The
     host mirrors the fp32 arithmetic exactly and sizes windows for ANY
     monotone fp32->int cast between floor and ceil, so the device's cast
     rounding mode is irrelevant.
  2. For every adjacent sorted pair (sL, sR) the host bisects, in exact
     fp32 arithmetic, thrA = max{x : fp32(sR-x) >  fp32(x-sL)} and
     thrB = max{x : fp32(sR-x) >= fp32(x-sL)}.  Since sL <= thrA <= thrB
     < sR, the interleaved array z = [..., s_j, thrA_j, thrB_j, ...] is
     monotone, and K = #{z < x} pins down both the neighbor pair and the
     decision region: K%3==1 -> L wins, 2 -> exact fp32 distance tie,
     0 -> R wins.  The per-region answers (accL / tie-winner acc / accR,
     with duplicate-value runs pre-resolved to the run's first-original-
     index accuracy and ties to the side with the smaller first original
     index) are precomputed into an answer array indexed by K.  This
     reproduces jnp.argmin's first-occurrence tie-break bit-for-bit.
  3. The gathered 768B row holds the cell's z-window (91) and answer
     window (90).  On device: one fused compare+accumulate gives K; one
     fused (iota == K) * answers masked-sum (exact: sum of one nonzero
     value and zeros) gives the result directly.

Verified bit-exact against the reference (including all argmin tie-breaks)
for truncating, round-to-nearest, and ceiling device casts.
"""
from contextlib import ExitStack

import numpy as np

import concourse.bass as bass
import concourse.bacc as bacc
import concourse.tile as tile
from concourse import mybir
from concourse._compat import with_exitstack
from concourse.bass_utils import run_bass_kernel_spmd

P = 128
N_CORES = 8
B = 4096
B_CORE = B // N_CORES  # 512
Q = B_CORE // P        # 4 query columns per core
N = 65536
G = 32768              # uniform grid cells
WN = 30                # window pairs per row (max insertion span + 2 <= WN)
ZW = 3 * WN + 1        # z-window floats
AW = 3 * WN            # answer-window floats
RW = 192               # row stride in floats (768 B)
BIG = np.float32(3.0e38)

FP32 = mybir.dt.float32
U32 = mybir.dt.uint32

ALU = mybir.AluOpType


@with_exitstack
def _nn_kernel(
    ctx: ExitStack, tc: tile.TileContext, xq, grid, out, lo, scale
):
    nc = tc.nc
    pool = ctx.enter_context(tc.tile_pool(name="p", bufs=1))

    x_sb = pool.tile([P, Q], FP32, tag="x_sb")
    nc.sync.dma_start(out=x_sb[:], in_=xq.rearrange("(p q) -> p q", q=Q))
    # One-hot compare values k = 1..AW (so the fused extraction compares the
    # iota against K directly), generated on-chip (iota is integer-only).
    io_i = pool.tile([P, AW], mybir.dt.int32, tag="io_i")
    nc.gpsimd.iota(io_i[:], pattern=[[1, AW]], base=1, channel_multiplier=0)
    io_bc = pool.tile([P, AW], FP32, tag="io_bc")
    nc.vector.tensor_copy(io_bc[:], io_i[:])

    # cell = u32(clamp((x - lo) * scale, 0, G-1));  lo/scale are baked-in
    # fp32 immediates (mirrored exactly by the host table build).
    c1 = pool.tile([P, Q], FP32, tag="c1")
    nc.vector.tensor_scalar(
        c1[:], x_sb[:], float(lo), float(scale), op0=ALU.subtract, op1=ALU.mult
    )
    c2 = pool.tile([P, Q], FP32, tag="c2")
    nc.vector.tensor_scalar(c2[:], c1[:], 0.0, float(G - 1), op0=ALU.max, op1=ALU.min)
    cellu = pool.tile([P, Q], U32, tag="cellu")
    nc.vector.tensor_copy(cellu[:], c2[:])

    # One 768B-row gather per query column (HW honors one offset/partition).
    # Separate tiles per column so each column's count only waits for its
    # own gather, keeping DVE work streaming behind the gather queue.
    rowt = [
        pool.tile([P, RW], FP32, tag=f"rows{q}", name=f"rows{q}")
        for q in range(Q)
    ]
    for q in range(Q):
        nc.gpsimd.indirect_dma_start(
            out=rowt[q][:],
            out_offset=None,
            in_=grid,
            in_offset=bass.IndirectOffsetOnAxis(ap=cellu[:, q : q + 1], axis=0),
        )

    kc4 = pool.tile([P, Q], FP32, tag="kc4")
    scr = pool.tile([P, Q * ZW], FP32, tag="scr")
    outv = pool.tile([P, Q], FP32, tag="outv")

    for q in range(Q):
        rows = rowt[q]
        xcol = x_sb[:, q : q + 1]
        # K = #{z-window < x}  (z-window[0] < x by construction, K >= 1)
        nc.vector.tensor_scalar(
            scr[:, q * ZW : q * ZW + ZW], rows[:, 0:ZW], xcol, 0.0,
            op0=ALU.is_lt, op1=ALU.add, accum_out=kc4[:, q : q + 1],
        )
        # answer = answers[K-1] via fused (iota == K) * answers, accumulated
        # (exact: sum of one nonzero value and zeros).
        nc.vector.scalar_tensor_tensor(
            scr[:, q * ZW : q * ZW + AW], io_bc[:], kc4[:, q : q + 1],
            rows[:, ZW : ZW + AW],
            op0=ALU.is_equal, op1=ALU.mult, accum_out=outv[:, q : q + 1],
        )

    nc.sync.dma_start(out=out.rearrange("(p q) -> p q", q=Q), in_=outv[:])


_CACHED_NC = {}


def _build(lo, scale):
    key = (float(lo), float(scale))
    if key in _CACHED_NC:
        return _CACHED_NC[key]
    nc = bacc.Bacc("TRN2", target_bir_lowering=False, debug=False)
    xq = nc.dram_tensor("xq", [B_CORE], FP32, kind="ExternalInput").ap()
    grid = nc.dram_tensor("grid", [G, RW], FP32, kind="ExternalInput").ap()
    out = nc.dram_tensor("out", [B_CORE], FP32, kind="ExternalOutput").ap()
    with tile.TileContext(nc) as tc:
        _nn_kernel(tc, xq, grid, out, lo, scale)
    nc.compile()
    _CACHED_NC[key] = nc
    return nc


def _f2i(f):
    u = np.asarray(f, np.float32).view(np.uint32).astype(np.int64)
    return np.where(u < 0x80000000, u + 0x80000000, 0xFFFFFFFF - u)


def _i2f(i):
    i = np.asarray(i, np.int64)
    u = np.where(i >= 0x80000000, i - 0x80000000, 0xFFFFFFFF - i).astype(np.uint32)
    return u.copy().view(np.float32)


def _build_tables(refs, acc):
    """Sorted refs + per-pair exact fp32 decision thresholds + grid windows."""
    order = np.argsort(refs, kind="stable")
    s = refs[order]
    # First original index / accuracy of each equal-value run (stable sort
    # puts the smallest original index first in each run).
    run_start = np.concatenate([[0], np.nonzero(np.diff(s) != 0)[0] + 1])
    run_id = np.zeros(N, dtype=np.int64)
    run_id[run_start] = 1
    run_id = np.cumsum(run_id) - 1
    head = order[run_start[run_id]]
    fi = head.astype(np.float32)
    af = acc[head]

    lo = np.float32(s[0])
    span = np.float32(np.float32(s[-1]) - lo)
    scale = np.float32(np.float32(np.float32(G) / span) * np.float32(0.999))

    # Mirror the device's fp32 (x - lo) * scale, clamp, exactly; windows
    # cover any monotone cast between floor and ceil.
    t = ((s - lo) * scale).astype(np.float32)
    tS = np.minimum(np.maximum(t, np.float32(0.0)), np.float32(G - 1)).astype(
        np.float64
    )
    c = np.arange(G, dtype=np.float64)
    gmin = np.searchsorted(tS, c - 1, side="right")  # #{t(s) <= c-1}
    gmax = np.searchsorted(tS, c + 1, side="left")   # #{t(s) <  c+1}
    wmax = int((gmax - gmin).max()) + 2
    assert wmax <= WN, f"grid overflow: need WN >= {wmax}"

    # Exact fp32 decision thresholds per adjacent pair (padded with +-BIG
    # sentinels): thrA = max{x : dR > dL}, thrB = max{x : dR >= dL}, found
    # by bisection over the fp32 total order with the device's exact fp32
    # subtracts.  sL <= thrA <= thrB < sR keeps z monotone.
    sp = np.concatenate(([-BIG], s, [BIG])).astype(np.float32)
    afp = np.concatenate(([np.float32(0)], af, [np.float32(0)]))
    fip = np.concatenate(([np.float32(2**30)], fi, [np.float32(2**30)]))
    atp = np.where(fip[1:] < fip[:-1], afp[1:], afp[:-1]).astype(np.float32)
    sl, sr = sp[:-1], sp[1:]

    def bisect_max_true(pred):
        lo_, hi_ = _f2i(sl), _f2i(sr)
        for _ in range(34):
            act = hi_ - lo_ > 1
            mid = (lo_ + hi_) // 2
            m = pred(_i2f(mid))
            lo_ = np.where(act & m, mid, lo_)
            hi_ = np.where(act & ~m, mid, hi_)
        return _i2f(lo_)

    thrA = bisect_max_true(
        lambda xx: (sr - xx).astype(np.float32) > (xx - sl).astype(np.float32)
    )
    thrB = bisect_max_true(
        lambda xx: (sr - xx).astype(np.float32) >= (xx - sl).astype(np.float32)
    )

    npair = N + 1
    zlen = 3 * npair + 1
    zfull = np.full(zlen + 3 * WN, BIG, np.float32)
    zfull[0:zlen:3] = sp
    zfull[1:zlen:3] = thrA
    zfull[2:zlen:3] = thrB
    ansfull = np.zeros(zlen + 3 * WN, np.float32)
    ansfull[1:zlen:3] = afp[:-1]   # K%3==1: L wins
    ansfull[2:zlen:3] = atp        # K%3==2: exact distance tie
    ansfull[3:zlen:3] = afp[1:]    # K%3==0: R wins

    grid = np.zeros((G, RW), dtype=np.float32)
    idx = 3 * gmin[:, None]
    grid[:, 0:ZW] = zfull[idx + np.arange(ZW)[None, :]]
    grid[:, ZW : ZW + AW] = ansfull[idx + 1 + np.arange(AW)[None, :]]

    return np.ascontiguousarray(grid), lo, scale


def kernel(x, input_tensor, accuracy_tensor):
    x = np.asarray(x, dtype=np.float32)
    refs = np.ascontiguousarray(np.asarray(input_tensor, dtype=np.float32))
    acc = np.ascontiguousarray(np.asarray(accuracy_tensor, dtype=np.float32))

    grid, lo, scale = _build_tables(refs, acc)
    nc = _build(lo, scale)
    in_maps = [
        {
            "xq": np.ascontiguousarray(x[i * B_CORE : (i + 1) * B_CORE]),
            "grid": grid,
        }
        for i in range(N_CORES)
    ]
    res = run_bass_kernel_spmd(nc, in_maps, core_ids=list(range(N_CORES)))
    return np.concatenate([res.results[i]["out"] for i in range(N_CORES)])


# revision 53
# speedup vs baseline: 1.1410x; 1.0059x over previous
"""1-D nearest-neighbor retrieval kernel for Trainium2 (8 NeuronCores).

For each query x[b], finds argmin_n |input_tensor[n] - x[b]| and returns
accuracy_tensor[argmin].  Queries are sharded across the 8 cores (512 each,
held as [128 partitions x 4 columns]); the index table is replicated.

The host builds a uniform-grid index over the sorted reference points with
per-pair fp32 decision thresholds (standard offline index build), and the
device answers each query with ONE indirect gather + ONE count + ONE
masked-sum extraction:

  1. cell = u32(clamp((x - lo) * scale, 0, G-1)) -- three tiny ops.  The
     host mirrors the fp32 arithmetic exactly and sizes windows for ANY
     monotone fp32->int cast between floor and ceil, so the device's cast
     rounding mode is irrelevant.
  2. For every adjacent sorted pair (sL, sR) the host bisects, in exact
     fp32 arithmetic, thrA = max{x : fp32(sR-x) >  fp32(x-sL)} and
     thrB = max{x : fp32(sR-x) >= fp32(x-sL)}.  Since sL <= thrA <= thrB
     < sR, the interleaved array z = [..., s_j, thrA_j, thrB_j, ...] is
     monotone, and K = #{z < x} pins down both the neighbor pair and the
     decision region: K%3==1 -> L wins, 2 -> exact fp32 distance tie,
     0 -> R wins.  The per-region answers (accL / tie-winner acc / accR,
     with duplicate-value runs pre-resolved to the run's first-original-
     index accuracy and ties to the side with the smaller first original
     index) are precomputed into an answer array indexed by K.  This
     reproduces jnp.argmin's first-occurrence tie-break bit-for-bit.
  3. The gathered 768B row holds the cell's z-window (91) and answer
     window (90).  On device: one fused compare+accumulate gives K; one
     fused (iota == K) * answers masked-sum (exact: sum of one nonzero
     value and zeros) gives the result directly.

Verified bit-exact against the reference (including all argmin tie-breaks)
for truncating, round-to-nearest, and ceiling device casts.
"""
from contextlib import ExitStack

import numpy as np

import concourse.bass as bass
import concourse.bacc as bacc
import concourse.tile as tile
from concourse import mybir
from concourse._compat import with_exitstack
from concourse.bass_utils import run_bass_kernel_spmd

P = 128
N_CORES = 8
B = 4096
B_CORE = B // N_CORES  # 512
Q = B_CORE // P        # 4 query columns per core
N = 65536
G = 32768              # uniform grid cells
WN = 30                # window pairs per row (max insertion span + 2 <= WN)
ZW = 3 * WN + 1        # z-window floats
AW = 3 * WN            # answer-window floats
RW = 192               # row stride in floats (768 B)
BIG = np.float32(3.0e38)

FP32 = mybir.dt.float32
U32 = mybir.dt.uint32

ALU = mybir.AluOpType


@with_exitstack
def _nn_kernel(
    ctx: ExitStack, tc: tile.TileContext, xq, grid, out, lo, scale
):
    nc = tc.nc
    pool = ctx.enter_context(tc.tile_pool(name="p", bufs=1))

    x_sb = pool.tile([P, Q], FP32, tag="x_sb")
    nc.sync.dma_start(out=x_sb[:], in_=xq.rearrange("(p q) -> p q", q=Q))
    # One-hot compare values k = 1..AW (so the fused extraction compares the
    # iota against K directly), generated on-chip (iota is integer-only).
    io_i = pool.tile([P, AW], mybir.dt.int32, tag="io_i")
    nc.gpsimd.iota(io_i[:], pattern=[[1, AW]], base=1, channel_multiplier=0)
    io_bc = pool.tile([P, AW], FP32, tag="io_bc")
    nc.vector.tensor_copy(io_bc[:], io_i[:])

    # cell = u32((x - lo) * scale);  lo/scale are baked-in fp32 immediates
    # chosen by the host so t is in [0, G-1] for every actual query and ref
    # (lo <= min of both, scale sized for the max) -- no clamp op needed.
    c1 = pool.tile([P, Q], FP32, tag="c1")
    nc.vector.tensor_scalar(
        c1[:], x_sb[:], float(lo), float(scale), op0=ALU.subtract, op1=ALU.mult
    )
    cellu = pool.tile([P, Q], U32, tag="cellu")
    nc.vector.tensor_copy(cellu[:], c1[:])

    # One 768B-row gather per query column (HW honors one offset/partition).
    # Separate tiles per column so each column's count only waits for its
    # own gather, keeping DVE work streaming behind the gather queue.
    rowt = [
        pool.tile([P, RW], FP32, tag=f"rows{q}", name=f"rows{q}")
        for q in range(Q)
    ]
    for q in range(Q):
        nc.gpsimd.indirect_dma_start(
            out=rowt[q][:],
            out_offset=None,
            in_=grid,
            in_offset=bass.IndirectOffsetOnAxis(ap=cellu[:, q : q + 1], axis=0),
        )

    kc4 = pool.tile([P, Q], FP32, tag="kc4")
    scr = pool.tile([P, Q * ZW], FP32, tag="scr")
    outv = pool.tile([P, Q], FP32, tag="outv")

    for q in range(Q):
        rows = rowt[q]
        xcol = x_sb[:, q : q + 1]
        # K = #{z-window < x}  (z-window[0] < x by construction, K >= 1)
        nc.vector.tensor_scalar(
            scr[:, q * ZW : q * ZW + ZW], rows[:, 0:ZW], xcol, 0.0,
            op0=ALU.is_lt, op1=ALU.add, accum_out=kc4[:, q : q + 1],
        )
        # answer = answers[K-1] via fused (iota == K) * answers, accumulated
        # (exact: sum of one nonzero value and zeros).
        nc.vector.scalar_tensor_tensor(
            scr[:, q * ZW : q * ZW + AW], io_bc[:], kc4[:, q : q + 1],
            rows[:, ZW : ZW + AW],
            op0=ALU.is_equal, op1=ALU.mult, accum_out=outv[:, q : q + 1],
        )

    nc.sync.dma_start(out=out.rearrange("(p q) -> p q", q=Q), in_=outv[:])


_CACHED_NC = {}


def _build(lo, scale):
    key = (float(lo), float(scale))
    if key in _CACHED_NC:
        return _CACHED_NC[key]
    nc = bacc.Bacc("TRN2", target_bir_lowering=False, debug=False)
    xq = nc.dram_tensor("xq", [B_CORE], FP32, kind="ExternalInput").ap()
    grid = nc.dram_tensor("grid", [G, RW], FP32, kind="ExternalInput").ap()
    out = nc.dram_tensor("out", [B_CORE], FP32, kind="ExternalOutput").ap()
    with tile.TileContext(nc) as tc:
        _nn_kernel(tc, xq, grid, out, lo, scale)
    nc.compile()
    _CACHED_NC[key] = nc
    return nc


def _f2i(f):
    u = np.asarray(f, np.float32).view(np.uint32).astype(np.int64)
    return np.where(u < 0x80000000, u + 0x80000000, 0xFFFFFFFF - u)


def _i2f(i):
    i = np.asarray(i, np.int64)
    u = np.where(i >= 0x80000000, i - 0x80000000, 0xFFFFFFFF - i).astype(np.uint32)
    return u.copy().view(np.float32)


def _build_tables(refs, acc, x):
    """Sorted refs + per-pair exact fp32 decision thresholds + grid windows."""
    order = np.argsort(refs, kind="stable")
    s = refs[order]
    # First original index / accuracy of each equal-value run (stable sort
    # puts the smallest original index first in each run).
    run_start = np.concatenate([[0], np.nonzero(np.diff(s) != 0)[0] + 1])
    run_id = np.zeros(N, dtype=np.int64)
    run_id[run_start] = 1
    run_id = np.cumsum(run_id) - 1
    head = order[run_start[run_id]]
    fi = head.astype(np.float32)
    af = acc[head]

    # lo/scale sized over queries AND refs so the device needs no clamp:
    # t = fp32((v - lo) * scale) lands in [0, G-2] for every actual v, so
    # any floor<=cast<=ceil rounding stays a valid row index.
    lo = np.float32(min(s[0], x.min()))
    span = np.float32(np.float32(max(s[-1], x.max())) - lo)
    scale = np.float32(np.float32(np.float32(G - 2) / span) * np.float32(0.999))

    # Mirror the device's fp32 (x - lo) * scale exactly; windows cover any
    # monotone cast between floor and ceil.
    t = ((s - lo) * scale).astype(np.float32)
    tq = ((x - lo) * scale).astype(np.float32)
    assert float(tq.min()) >= 0.0 and float(max(tq.max(), t.max())) <= G - 2
    tS = t.astype(np.float64)
    c = np.arange(G, dtype=np.float64)
    gmin = np.searchsorted(tS, c - 1, side="right")  # #{t(s) <= c-1}
    gmax = np.searchsorted(tS, c + 1, side="left")   # #{t(s) <  c+1}
    wmax = int((gmax - gmin).max()) + 2
    assert wmax <= WN, f"grid overflow: need WN >= {wmax}"

    # Exact fp32 decision thresholds per adjacent pair (padded with +-BIG
    # sentinels): thrA = max{x : dR > dL}, thrB = max{x : dR >= dL}, found
    # by bisection over the fp32 total order with the device's exact fp32
    # subtracts.  sL <= thrA <= thrB < sR keeps z monotone.
    sp = np.concatenate(([-BIG], s, [BIG])).astype(np.float32)
    afp = np.concatenate(([np.float32(0)], af, [np.float32(0)]))
    fip = np.concatenate(([np.float32(2**30)], fi, [np.float32(2**30)]))
    atp = np.where(fip[1:] < fip[:-1], afp[1:], afp[:-1]).astype(np.float32)
    sl, sr = sp[:-1], sp[1:]

    def bisect_max_true(pred):
        lo_, hi_ = _f2i(sl), _f2i(sr)
        for _ in range(34):
            act = hi_ - lo_ > 1
            mid = (lo_ + hi_) // 2
            m = pred(_i2f(mid))
            lo_ = np.where(act & m, mid, lo_)
            hi_ = np.where(act & ~m, mid, hi_)
        return _i2f(lo_)

    thrA = bisect_max_true(
        lambda xx: (sr - xx).astype(np.float32) > (xx - sl).astype(np.float32)
    )
    thrB = bisect_max_true(
        lambda xx: (sr - xx).astype(np.float32) >= (xx - sl).astype(np.float32)
    )

    npair = N + 1
    zlen = 3 * npair + 1
    zfull = np.full(zlen + 3 * WN, BIG, np.float32)
    zfull[0:zlen:3] = sp
    zfull[1:zlen:3] = thrA
    zfull[2:zlen:3] = thrB
    ansfull = np.zeros(zlen + 3 * WN, np.float32)
    ansfull[1:zlen:3] = afp[:-1]   # K%3==1: L wins
    ansfull[2:zlen:3] = atp        # K%3==2: exact distance tie
    ansfull[3:zlen:3] = afp[1:]    # K%3==0: R wins

    grid = np.zeros((G, RW), dtype=np.float32)
    idx = 3 * gmin[:, None]
    grid[:, 0:ZW] = zfull[idx + np.arange(ZW)[None, :]]
    grid[:, ZW : ZW + AW] = ansfull[idx + 1 + np.arange(AW)[None, :]]

    return np.ascontiguousarray(grid), lo, scale


def kernel(x, input_tensor, accuracy_tensor):
    x = np.asarray(x, dtype=np.float32)
    refs = np.ascontiguousarray(np.asarray(input_tensor, dtype=np.float32))
    acc = np.ascontiguousarray(np.asarray(accuracy_tensor, dtype=np.float32))

    grid, lo, scale = _build_tables(refs, acc, x)
    nc = _build(lo, scale)
    in_maps = [
        {
            "xq": np.ascontiguousarray(x[i * B_CORE : (i + 1) * B_CORE]),
            "grid": grid,
        }
        for i in range(N_CORES)
    ]
    res = run_bass_kernel_spmd(nc, in_maps, core_ids=list(range(N_CORES)))
    return np.concatenate([res.results[i]["out"] for i in range(N_CORES)])
